# revision 23
# baseline (speedup 1.0000x reference)
"""Trainium2 Bass kernel for the border-ownership / grouping spiking model.

Pipeline (per 512x512 image, 2 polarity channels):
  conv1: 8 filters 11x11 on each polarity (pad 5)  -> spike (>=1)
  elementwise border-ownership logic (exact small-int algebra)
  conv2: depthwise 23x23 over 16 border channels (pad 11) -> spike
  orientation combine -> [B, H, W] output

Sharding: 8 cores = 4 images x 2 row-halves (256 rows each), halo
recomputed locally (16 input rows each side).

v3 design (vs the fp16 baseline):
  - conv1 in fp8(e4m3) with DoubleRow: two horizontal taps per matmul
    (banded-Toeplitz pairs; x duplicated in SBUF at a 16-aligned column
    offset so the pair AP step is legal).  Zero threshold flips verified
    against exact f64 on this model (margin 0.141; quantization errors
    are relative, the threshold is absolute at 1.0).
  - elementwise logic batched over the 4 orientations in [rows, 2048]
    tiles; diff = (pe+ne) - (po+no) exactly (the inhibition product
    terms cancel), w1 folded into the WTA gate, border products emitted
    with fused scalar_tensor_tensor compares.
  - border planes stored channel-major in one tile per conv1 row-tile;
    conv2 reads them directly with 2-way contract-split matmuls (no
    E-tile assembly DMAs).
  - conv2 is skipped per (channel-pair, row-span) via on-device
    all-zero flags + tc.If (exact for any input: conv of zeros is zero,
    spike(0)=0, and the pair combine a*(1-g) vanishes with a==0).
  - the 8 unique 23x23 group-filter bands are loaded to SBUF once,
    outside the timing loop.
"""

import os
from contextlib import nullcontext
import numpy as np
import ml_dtypes

import concourse.bass as bass
import concourse.tile as tile
from concourse import bacc, mybir
from concourse.ap import AP
from concourse.bass_utils import run_bass_kernel_spmd
from concourse.alu_op_type import AluOpType

USE_SKIP = os.environ.get("K_SKIP", "1") == "1"
STAGE = int(os.environ.get("K_STAGE", "4"))

N_CORES = 8
H = W = 512
HALF = 256
BK, GK = 11, 23  # kernel sizes
PB, PG = 5, 11   # paddings

# conv1 tiling: OVERLAPPING row-tiles (bases 0/96/192) so that each
# conv2 out-tile's contract rows live wholly in one border tile at
# partition base 0 (matmul operands must start at partition 0/32/64).
# Out rows per core: 256 + 2*11 halo = 278; computed rows 118+118+86.
C1_BASE = [0, 96, 192]
C1_OUT = [118, 118, 86]
C1_IN = [128, 128, 96]
C1_ROWS = 278
# conv2 output tiling of the core's 256 rows; out-tile e reads contract
# rows [0, E_IN[e]) of border tile e directly.
E_BASE = [0, 96, 192]
E_OUT = [96, 96, 64]
E_IN = [118, 118, 86]

XW = W + BK - 1          # 522 input cols (x-halo +-5)
BW = W + GK - 1          # 534 border cols (x-halo +-11)
IN_ROWS = 288            # input rows per core ([start-16, start+272))
OW = 4 * W               # 2048: orientation-batched tile width

# fp8 DoubleRow layout: x stored twice in SBUF, copy B at column DUP
# so that pair (dx, dx+1) has AP step DUP+1 (= 544, 16-aligned).
DUP = 543
X8W = 1072               # SBUF x tile width (543 + 522 = 1065, pad)
X8DW = 544               # DRAM x row width (522 data + pad)
NPAIR = 6                # 11 dx taps -> 5 pairs + 1 single (B weights 0)

f8 = mybir.dt.float8e4
f16 = mybir.dt.float16
bf16 = mybir.dt.bfloat16
f32 = mybir.dt.float32
i32 = mybir.dt.int32
e4m3 = ml_dtypes.float8_e4m3fn
ET = mybir.EngineType
AX = mybir.AxisListType


def _band(wcol, K, M):
    """Banded Toeplitz lhsT [K, M]: band[k, m] = wcol[k - m]."""
    out = np.zeros((K, M), dtype=wcol.dtype)
    for j in range(len(wcol)):
        idx = np.arange(0, min(M, K - j))
        out[idx + j, idx] = wcol[j]
    return out


def _make_bands(W_border, W_group):
    Wb8 = np.asarray(W_border, dtype=np.float32).reshape(8, BK, BK).astype(e4m3)
    Wg16 = np.asarray(W_group, dtype=np.float32).reshape(16, GK, GK).astype(np.float16)
    # conv1 DoubleRow bands: [128, 8*6*256] fp8.
    # block (ch, p): band(dx=2p) at cols [0:118], band(dx=2p+1) at [128:246]
    bandsB = np.zeros((128, 8 * NPAIR * 256), dtype=e4m3)
    for ch in range(8):
        for p in range(NPAIR):
            base = (ch * NPAIR + p) * 256
            bandsB[:, base:base + 118] = _band(Wb8[ch, :, 2 * p], 128, 118)
            if 2 * p + 1 < BK:
                bandsB[:, base + 128:base + 246] = _band(Wb8[ch, :, 2 * p + 1], 128, 118)
    # conv2 bands: 8 unique filters (channels 4o+0==4o+1, 4o+2==4o+3),
    # unique index u = 2*o + pk maps to channel 4*o + 2*pk.
    bandsG = np.zeros((8, 128, GK * 106), dtype=np.float16)
    for u in range(8):
        o, pk = divmod(u, 2)
        ch = 4 * o + 2 * pk
        for dx in range(GK):
            bandsG[u, :, dx * 106:(dx + 1) * 106] = _band(Wg16[ch, :, dx], 128, 106)
    return bandsB, bandsG


def _prep_inputs(inp):
    inp = np.asarray(inp, dtype=np.float32)
    inp16 = inp.astype(np.float16)
    in_maps = []
    for r in range(N_CORES):
        b, half = divmod(r, 2)
        start = HALF * half
        # x8: fp8 [2, 288, 544], rows = image[start-16, start+272), cols [-5, 517)
        x8 = np.zeros((2, IN_ROWS, X8DW), dtype=e4m3)
        r0, r1 = start - 16, start + 272
        sr0, sr1 = max(r0, 0), min(r1, H)
        x8[:, sr0 - r0:sr1 - r0, PB:PB + W] = inp16[b, :, sr0:sr1, :].astype(e4m3)
        # vmap: f32 [278, 512], rows = image[start-11, start+267)
        vm = np.zeros((C1_ROWS, W), dtype=np.float32)
        v0, v1 = start - 11, start + 267
        sv0, sv1 = max(v0, 0), min(v1, H)
        vm[sv0 - v0:sv1 - v0] = inp[b, 0, sv0:sv1] + inp[b, 1, sv0:sv1]
        in_maps.append({"x8": x8, "vmap": vm})
    return in_maps


def _c1_lhsT(bands_t, ch, p, K, M):
    """DoubleRow lhsT AP [K, 2, M] on a [128, 8*6*256] band tile."""
    base = (ch * NPAIR + p) * 256
    ap = bands_t[0:K, base:base + 256]
    return AP(ap.tensor, ap.offset, [[8 * NPAIR * 256, K], [128, 2], [1, M]])


def _c1_rhs(x_t, p, K):
    """DoubleRow rhs AP [K, 2, 512] on a [128, X8W] duplicated x tile."""
    ap = x_t[0:K, 2 * p:2 * p + W]
    return AP(ap.tensor, ap.offset, [[X8W, K], [DUP + 1, 2], [1, W]])


def _obatch(tl, rows):
    """AP [rows, 4, 512] over the o-major blocks of a [rows, 2048] tile."""
    ap = tl[0:rows, 0:1]
    return AP(ap.tensor, ap.offset, [[OW, rows], [W, 4], [1, W]])


def _emit(nc, tc, ctx, x8_d, vmap_d, bandsB_t, bandsG_t, out_d):
    x_pool = ctx.enter_context(tc.tile_pool(name="x", bufs=2))
    spk_pool = ctx.enter_context(tc.tile_pool(name="spk", bufs=2))
    brd_pool = ctx.enter_context(tc.tile_pool(name="brd", bufs=1))
    tmp_pool = ctx.enter_context(tc.tile_pool(name="tmp", bufs=1))
    vm_pool = ctx.enter_context(tc.tile_pool(name="vm", bufs=1))
    acc_pool = ctx.enter_context(tc.tile_pool(name="acc", bufs=1))
    fl_pool = ctx.enter_context(tc.tile_pool(name="fl", bufs=1))
    oacc_pool = ctx.enter_context(tc.tile_pool(name="oacc", bufs=1))
    c2_pool = ctx.enter_context(tc.tile_pool(name="c2", bufs=1))
    ps1 = ctx.enter_context(tc.tile_pool(name="ps1", bufs=3, space="PSUM"))
    psf = ctx.enter_context(tc.tile_pool(name="psf", bufs=1, space="PSUM"))
    ps2 = ctx.enter_context(tc.tile_pool(name="ps2", bufs=2, space="PSUM"))

    def mk(pool, shape, dtype, tag):
        return pool.tile(shape, dtype, tag=tag, name=tag)

    # channel-major border tiles, one per conv1 row-tile
    borderT = [mk(brd_pool, [C1_OUT[t], 16 * BW], f16, f"bT{t}")
               for t in range(3)]

    ones_c = mk(fl_pool, [128, 1], f32, "ones")
    nc.vector.memset(ones_c[:, :], 1.0)
    flp = [mk(psf, [1, 16], f32, f"flp{t}") for t in range(3)]
    flags_i = mk(fl_pool, [1, 16], i32, "flagsi")
    fall = mk(fl_pool, [1, 16], f32, "fall")
    fsb = mk(fl_pool, [1, 48], f32, "fsb")

    # ---- per conv1 tile: conv1 (fp8 DoubleRow), spikes, border logic ------
    for t in range(3 if STAGE >= 1 else 0):
        rows = C1_OUT[t]
        # orientation-batched spike tiles: slice o at cols [512o, 512o+512)
        SPE = mk(spk_pool, [rows, OW], bf16, "SPE")  # pol0 even ch (pe)
        SPO = mk(spk_pool, [rows, OW], bf16, "SPO")  # pol0 odd ch (po)
        SNE = mk(spk_pool, [rows, OW], bf16, "SNE")  # pol1 even ch (ne)
        SNO = mk(spk_pool, [rows, OW], bf16, "SNO")  # pol1 odd ch (no)
        CPO = mk(spk_pool, [rows, OW], bf16, "CPO")  # conv values via ACT
        CNO = mk(spk_pool, [rows, OW], bf16, "CNO")

        def spike_from(psum_ap, ch, pol, rows=None):
            o2, par = divmod(ch, 2)
            sl = slice(W * o2, W * o2 + W)
            if par == 0:
                dst = SPE if pol == 0 else SNE
                nc.vector.tensor_single_scalar(dst[:, sl], psum_ap, 1.0,
                                               AluOpType.is_ge)
            else:
                # odd channels: ACT copies the conv values, compare later
                cdst = CPO if pol == 0 else CNO
                nc.scalar.copy(cdst[:, sl], psum_ap)

        xt = []
        for pol in range(2):
            xx = mk(x_pool, [C1_IN[t], X8W], f8, f"x{pol}")
            nc.gpsimd.memset(xx[:, :], 0.0)
            nc.sync.dma_start(
                xx[:, 0:XW],
                x8_d[pol, C1_BASE[t]:C1_BASE[t] + C1_IN[t], 0:XW])
            nc.sync.dma_start(
                xx[:, DUP:DUP + XW],
                x8_d[pol, C1_BASE[t]:C1_BASE[t] + C1_IN[t], 0:XW])
            xt.append(xx)
        K = C1_IN[t]
        for ch in range(8):
            pp = [mk(ps1, [rows, W], f32, "c1") for _ in range(2)]
            for p in range(NPAIR):
                lhsT = _c1_lhsT(bandsB_t, ch, p, K, rows)
                for pol in range(2):
                    nc.tensor.matmul(
                        pp[pol][:, :], lhsT, _c1_rhs(xt[pol], p, K),
                        start=(p == 0), stop=(p == NPAIR - 1),
                        perf_mode=mybir.MatmulPerfMode.DoubleRow)
            for pol in range(2):
                spike_from(pp[pol][:, :], ch, pol)

        # batched compares for the odd channels
        nc.vector.tensor_single_scalar(SPO[:, :], CPO[:, :], 1.0,
                                       AluOpType.is_ge)
        nc.vector.tensor_single_scalar(SNO[:, :], CNO[:, :], 1.0,
                                       AluOpType.is_ge)

        if STAGE < 2:
            continue

        vm_t = mk(vm_pool, [rows, W], f32, "vm")
        nc.sync.dma_start(vm_t[:], vmap_d[C1_BASE[t]:C1_BASE[t] + rows, :])
        w1 = mk(vm_pool, [rows, W], bf16, "w1")
        nc.gpsimd.tensor_single_scalar(w1[:], vm_t[:], 1.0, AluOpType.is_ge)

        # zero the x-halo edges of all 16 border planes (strided memsets)
        bT = borderT[t]
        nc.gpsimd.memset(
            AP(bT.tensor, bT[0:rows, 0:1].offset,
               [[16 * BW, rows], [BW, 16], [1, PG]]), 0.0)
        nc.gpsimd.memset(
            AP(bT.tensor, bT[0:rows, 0:1].offset + PG + W,
               [[16 * BW, rows], [BW, 16], [1, PG]]), 0.0)

        def T(tag):
            return mk(tmp_pool, [rows, OW], bf16, tag)

        # exact algebra, orientation-batched:
        #   e13 = pe+ne - (pe*no + ne*po); e24 = po+no - (pe*no + ne*po)
        #   diff = e13 - e24 = (pe+ne) - (po+no)   (products cancel)
        A1 = T("A1"); nc.vector.tensor_mul(A1[:], SPE[:], SNO[:])
        B1 = T("B1"); nc.vector.tensor_mul(B1[:], SNE[:], SPO[:])
        C1 = T("C1"); nc.vector.tensor_add(C1[:], SPE[:], SNE[:])
        C2 = T("C2"); nc.vector.tensor_add(C2[:], SPO[:], SNO[:])
        DIFF = T("DIFF"); nc.vector.tensor_sub(DIFF[:], C1[:], C2[:])
        D1 = T("D1"); nc.vector.tensor_add(D1[:], A1[:], B1[:])
        E13 = T("E13"); nc.vector.tensor_sub(E13[:], C1[:], D1[:])
        E24 = T("E24"); nc.vector.tensor_sub(E24[:], C2[:], D1[:])
        TP = T("TP")
        nc.scalar.activation(TP[:], DIFF[:], mybir.ActivationFunctionType.Abs)
        # tmax over the 4 orientations
        m01 = mk(tmp_pool, [rows, W], bf16, "m01")
        nc.vector.tensor_max(m01[:], TP[:, 0:W], TP[:, W:2 * W])
        m23 = mk(tmp_pool, [rows, W], bf16, "m23")
        nc.vector.tensor_max(m23[:], TP[:, 2 * W:3 * W], TP[:, 3 * W:4 * W])
        TMAX = mk(tmp_pool, [rows, W], bf16, "TMAX")
        nc.vector.tensor_max(TMAX[:], m01[:], m23[:])
        # wta gate with w1 folded in: wd2 = (tp==tmax)*diff*w1
        tmaxb = AP(TMAX.tensor, TMAX[0:rows, 0:1].offset,
                   [[W, rows], [0, 4], [1, W]])
        w1b = AP(w1.tensor, w1[0:rows, 0:1].offset,
                 [[W, rows], [0, 4], [1, W]])
        WTA = T("C2")  # reuse: C2 dead after E24
        nc.vector.tensor_tensor(WTA[:], TP[:], tmaxb, AluOpType.is_equal)
        WD = T("D1")   # reuse: D1 dead after E24
        nc.vector.tensor_mul(WD[:], WTA[:], DIFF[:])
        # winner gates with the vmap mask folded in:
        # b1p = (wd>=1)*w1, b1n = (wd<=-1)*w1   (w1 in {0,1})
        B1P = T("C1")  # reuse: C1 dead after E13
        nc.vector.scalar_tensor_tensor(B1P[:], WD[:], 1.0, w1b,
                                       AluOpType.is_ge, AluOpType.mult)
        B1N = T("A1")  # reuse: A1 dead after D1
        nc.vector.scalar_tensor_tensor(B1N[:], WD[:], -1.0, w1b,
                                       AluOpType.is_le, AluOpType.mult)
        # border products with fused per-partition plane sums (the zero
        # flags): plane ch=4o+k at cols ch*534 + [11, 523), acc col ch.
        acc_t = mk(acc_pool, [rows, 16], f32, "acc")
        for o in range(4):
            sl = slice(W * o, W * o + W)
            for k, (gate, src) in enumerate([
                    (B1P, E13), (B1P, E24), (B1N, E24), (B1N, E13)]):
                ch = 4 * o + k
                nc.vector.scalar_tensor_tensor(
                    bT[0:rows, ch * BW + PG:ch * BW + PG + W],
                    gate[:, sl], 1.0, src[:, sl],
                    AluOpType.mult, AluOpType.mult,
                    accum_out=acc_t[:, ch:ch + 1])

        if STAGE < 3:
            continue
        nc.tensor.matmul(flp[t][0:1, 0:16], ones_c[0:rows, :],
                         acc_t[:, :], start=True, stop=True)

    # ---- flags: per-plane nonzero counts; plane index 2q+j is already
    # (pair q, channel j), so the pair layout is the identity.
    if STAGE >= 3:
        for t in range(3):
            nc.scalar.copy(fsb[0:1, 16 * t:16 * t + 16], flp[t][0:1, :])
        nc.vector.tensor_add(fall[0:1, :], fsb[0:1, 0:16], fsb[0:1, 16:32])
        nc.vector.tensor_add(fall[0:1, :], fall[0:1, :], fsb[0:1, 32:48])
        nc.vector.tensor_copy(flags_i[0:1, :], fall[0:1, :])

    # ---- conv2, skipped per (pair, span) when the a-plane is all zero -----
    oacc = [mk(oacc_pool, [E_OUT[e], W], f32, f"oacc{e}") for e in range(3)]
    for e in range(3):
        nc.gpsimd.memset(oacc[e][:, :], 0.0)

    def _c2conv(ch, e, u, tag):
        """spike(conv2) of border plane ch on out-tile e with band u."""
        orows, krows = E_OUT[e], E_IN[e]
        pg = mk(ps2, [orows, W], f32, "c2")
        gb = bandsG_t[u]
        for dx in range(GK):
            nc.tensor.matmul(
                pg[:, :],
                gb[0:krows, dx * 106:dx * 106 + orows],
                borderT[e][0:krows, ch * BW + dx:ch * BW + dx + W],
                start=(dx == 0), stop=(dx == GK - 1))
        s = mk(c2_pool, [orows, W], bf16, tag)
        nc.vector.tensor_single_scalar(s[:], pg[:, :], 1.0, AluOpType.is_ge)
        return s

    if STAGE >= 4 and USE_SKIP:
        _, pair_flags = nc.values_load_multi_w_load_instructions(
            flags_i[0:1, 0:16], engines=[ET.PE, ET.DVE],
            skip_runtime_bounds_check=True)
    for q in range(8 if STAGE >= 4 else 0):
        o, pk = divmod(q, 2)
        ch0 = 4 * o + 2 * pk
        u = 2 * o + pk

        def IF(c, nm):
            return tc.If(c, name=nm) if USE_SKIP else nullcontext()
        with IF(pair_flags[2 * q] != 0 if USE_SKIP else 1, f"q{q}"):
            aa = []
            for e in range(3):
                a = _c2conv(ch0, e, u, f"a{e}")
                nc.vector.tensor_add(oacc[e][:, :], oacc[e][:, :], a[:])
                aa.append(a)
            with IF(pair_flags[2 * q + 1] != 0 if USE_SKIP else 1, f"qn{q}"):
                for e in range(3):
                    g = _c2conv(ch0 + 1, e, u, "g")
                    ag = mk(c2_pool, [E_OUT[0], W], bf16, "ag")[0:E_OUT[e], :]
                    nc.vector.tensor_mul(ag[:], aa[e][:], g[:])
                    nc.vector.tensor_sub(oacc[e][:, :], oacc[e][:, :], ag[:])

    for e in range(3):
        nc.sync.dma_start(out_d[E_BASE[e]:E_BASE[e] + E_OUT[e], :], oacc[e][:])


def _build_program(bandsB_np, bandsG_np, reps=1):
    from contextlib import ExitStack
    nc = bacc.Bacc("TRN2", target_bir_lowering=False, debug=False,
                   num_devices=N_CORES)
    x8_d = nc.dram_tensor("x8", [2, IN_ROWS, X8DW], f8, kind="ExternalInput").ap()
    vmap_d = nc.dram_tensor("vmap", [C1_ROWS, W], f32, kind="ExternalInput").ap()
    bandsB_d = nc.inline_tensor(bandsB_np, name="bandsB").ap()
    bandsG_d = nc.inline_tensor(bandsG_np, name="bandsG").ap()
    out_d = nc.dram_tensor("out", [HALF, W], f32, kind="ExternalOutput").ap()

    with tile.TileContext(nc) as tc:
        with ExitStack() as octx:
            band_pool = octx.enter_context(tc.tile_pool(name="bands", bufs=1))
            bandsB_t = band_pool.tile([128, 8 * NPAIR * 256], f8,
                                      tag="bB", name="bB")
            nc.sync.dma_start(bandsB_t[:], bandsB_d)
            bandsG_t = []
            for u in range(8):
                g = band_pool.tile([128, GK * 106], f16, tag=f"bG{u}",
                                   name=f"bG{u}")
                nc.sync.dma_start(g[:], bandsG_d[u])
                bandsG_t.append(g)
            if reps == 1:
                with ExitStack() as ctx:
                    _emit(nc, tc, ctx, x8_d, vmap_d, bandsB_t,
                          bandsG_t, out_d)
            else:
                with tc.For_i(0, reps, 1,
                              staggered_reset=os.environ.get("K_SR", "1") == "1"):
                    with ExitStack() as ctx:
                        _emit(nc, tc, ctx, x8_d, vmap_d, bandsB_t,
                              bandsG_t, out_d)
    nc.compile()
    return nc


_PROGRAM_CACHE = {}


def kernel(inp, W_border, W_group):
    in_maps = _prep_inputs(inp)
    bandsB_np, bandsG_np = _make_bands(W_border, W_group)
    key = (bandsB_np.tobytes(), bandsG_np.tobytes())
    if _PROGRAM_CACHE.get("key") != key:
        _PROGRAM_CACHE["nc"] = _build_program(bandsB_np, bandsG_np)
        _PROGRAM_CACHE["key"] = key
    res = run_bass_kernel_spmd(_PROGRAM_CACHE["nc"], in_maps, list(range(N_CORES)))
    out = np.empty((4, H, W), dtype=np.float32)
    for r in range(N_CORES):
        b, half = divmod(r, 2)
        out[b, HALF * half:HALF * (half + 1), :] = res.results[r]["out"]
    return out


# revision 24
# speedup vs baseline: 1.8818x; 1.8818x over previous
"""Trainium2 Bass kernel for the border-ownership / grouping spiking model.

Pipeline (per 512x512 image, 2 polarity channels):
  conv1: 8 filters 11x11 on each polarity (pad 5)  -> spike (>=1)
  elementwise border-ownership logic (exact small-int algebra)
  conv2: depthwise 23x23 over 16 border channels (pad 11) -> spike
  orientation combine -> [B, H, W] output

Sharding: 8 cores = 4 images x 2 row-halves (256 rows each), halo
recomputed locally (16 input rows each side).

v3 design (vs the fp16 baseline):
  - conv1 in fp8(e4m3) with DoubleRow: two horizontal taps per matmul
    (banded-Toeplitz pairs; x duplicated in SBUF at a 16-aligned column
    offset so the pair AP step is legal).  Zero threshold flips verified
    against exact f64 on this model (margin 0.141; quantization errors
    are relative, the threshold is absolute at 1.0).
  - elementwise logic batched over the 4 orientations in [rows, 2048]
    tiles; diff = (pe+ne) - (po+no) exactly (the inhibition product
    terms cancel), w1 folded into the WTA gate, border products emitted
    with fused scalar_tensor_tensor compares.
  - border planes stored channel-major in one tile per conv1 row-tile;
    conv2 reads them directly with 2-way contract-split matmuls (no
    E-tile assembly DMAs).
  - conv2 is skipped per (channel-pair, row-span) via on-device
    all-zero flags + tc.If (exact for any input: conv of zeros is zero,
    spike(0)=0, and the pair combine a*(1-g) vanishes with a==0).
  - the 8 unique 23x23 group-filter bands are loaded to SBUF once,
    outside the timing loop.
"""

import os
from contextlib import nullcontext
import numpy as np
import ml_dtypes

import concourse.bass as bass
import concourse.tile as tile
from concourse import bacc, mybir
from concourse.ap import AP
from concourse.bass_utils import run_bass_kernel_spmd
from concourse.alu_op_type import AluOpType

USE_SKIP = os.environ.get("K_SKIP", "1") == "1"
STAGE = int(os.environ.get("K_STAGE", "4"))

N_CORES = 8
H = W = 512
HALF = 256
BK, GK = 11, 23  # kernel sizes
PB, PG = 5, 11   # paddings

# conv1 tiling: OVERLAPPING row-tiles (bases 0/96/192) so that each
# conv2 out-tile's contract rows live wholly in one border tile at
# partition base 0 (matmul operands must start at partition 0/32/64).
# Out rows per core: 256 + 2*11 halo = 278; computed rows 118+118+86.
C1_BASE = [0, 96, 192]
C1_OUT = [118, 118, 86]
C1_IN = [128, 128, 96]
C1_ROWS = 278
# conv2 output tiling of the core's 256 rows; out-tile e reads contract
# rows [0, E_IN[e]) of border tile e directly.
E_BASE = [0, 96, 192]
E_OUT = [96, 96, 64]
E_IN = [118, 118, 86]

XW = W + BK - 1          # 522 input cols (x-halo +-5)
BW = W + GK - 1          # 534 border cols (x-halo +-11)
IN_ROWS = 288            # input rows per core ([start-16, start+272))
OW = 4 * W               # 2048: orientation-batched tile width

# fp8 DoubleRow layout: x stored twice in SBUF, copy B at column DUP
# so that pair (dx, dx+1) has AP step DUP+1 (= 544, 16-aligned).
DUP = 543
X8W = 1072               # SBUF x tile width (543 + 522 = 1065, pad)
X8DW = 544               # DRAM x row width (522 data + pad)
NPAIR = 6                # 11 dx taps -> 5 pairs + 1 single (B weights 0)

f8 = mybir.dt.float8e4
f16 = mybir.dt.float16
bf16 = mybir.dt.bfloat16
f32 = mybir.dt.float32
i32 = mybir.dt.int32
e4m3 = ml_dtypes.float8_e4m3fn
ET = mybir.EngineType
AX = mybir.AxisListType


def _band(wcol, K, M):
    """Banded Toeplitz lhsT [K, M]: band[k, m] = wcol[k - m]."""
    out = np.zeros((K, M), dtype=wcol.dtype)
    for j in range(len(wcol)):
        idx = np.arange(0, min(M, K - j))
        out[idx + j, idx] = wcol[j]
    return out


def _make_bands(W_border, W_group):
    Wb8 = np.asarray(W_border, dtype=np.float32).reshape(8, BK, BK).astype(e4m3)
    Wg16 = np.asarray(W_group, dtype=np.float32).reshape(16, GK, GK).astype(np.float16)
    # conv1 DoubleRow bands: [128, 8*6*256] fp8.
    # block (ch, p): band(dx=2p) at cols [0:118], band(dx=2p+1) at [128:246]
    bandsB = np.zeros((128, 8 * NPAIR * 256), dtype=e4m3)
    for ch in range(8):
        for p in range(NPAIR):
            base = (ch * NPAIR + p) * 256
            bandsB[:, base:base + 118] = _band(Wb8[ch, :, 2 * p], 128, 118)
            if 2 * p + 1 < BK:
                bandsB[:, base + 128:base + 246] = _band(Wb8[ch, :, 2 * p + 1], 128, 118)
    # conv2 bands: 8 unique filters (channels 4o+0==4o+1, 4o+2==4o+3),
    # unique index u = 2*o + pk maps to channel 4*o + 2*pk.
    bandsG = np.zeros((8, 128, GK * 106), dtype=np.float16)
    for u in range(8):
        o, pk = divmod(u, 2)
        ch = 4 * o + 2 * pk
        for dx in range(GK):
            bandsG[u, :, dx * 106:(dx + 1) * 106] = _band(Wg16[ch, :, dx], 128, 106)
    return bandsB, bandsG


def _prep_inputs(inp):
    inp = np.asarray(inp, dtype=np.float32)
    inp16 = inp.astype(np.float16)
    in_maps = []
    for r in range(N_CORES):
        b, half = divmod(r, 2)
        start = HALF * half
        # x8: fp8 [2, 288, 544], rows = image[start-16, start+272), cols [-5, 517)
        x8 = np.zeros((2, IN_ROWS, X8DW), dtype=e4m3)
        r0, r1 = start - 16, start + 272
        sr0, sr1 = max(r0, 0), min(r1, H)
        x8[:, sr0 - r0:sr1 - r0, PB:PB + W] = inp16[b, :, sr0:sr1, :].astype(e4m3)
        # vmap: f32 [278, 512], rows = image[start-11, start+267)
        vm = np.zeros((C1_ROWS, W), dtype=np.float32)
        v0, v1 = start - 11, start + 267
        sv0, sv1 = max(v0, 0), min(v1, H)
        vm[sv0 - v0:sv1 - v0] = inp[b, 0, sv0:sv1] + inp[b, 1, sv0:sv1]
        in_maps.append({"x8": x8, "vmap": vm})
    return in_maps


def _c1_lhsT(bands_t, ch, p, K, M):
    """DoubleRow lhsT AP [K, 2, M] on a [128, 8*6*256] band tile."""
    base = (ch * NPAIR + p) * 256
    ap = bands_t[0:K, base:base + 256]
    return AP(ap.tensor, ap.offset, [[8 * NPAIR * 256, K], [128, 2], [1, M]])


def _c1_rhs(x_t, p, K):
    """DoubleRow rhs AP [K, 2, 512] on a [128, X8W] duplicated x tile."""
    ap = x_t[0:K, 2 * p:2 * p + W]
    return AP(ap.tensor, ap.offset, [[X8W, K], [DUP + 1, 2], [1, W]])


def _obatch(tl, rows):
    """AP [rows, 4, 512] over the o-major blocks of a [rows, 2048] tile."""
    ap = tl[0:rows, 0:1]
    return AP(ap.tensor, ap.offset, [[OW, rows], [W, 4], [1, W]])


def _emit(nc, tc, ctx, x8_d, vmap_d, bandsB_t, bandsG_t, out_d):
    x_pool = ctx.enter_context(tc.tile_pool(name="x", bufs=2))
    spk_pool = ctx.enter_context(tc.tile_pool(name="spk", bufs=2))
    brd_pool = ctx.enter_context(tc.tile_pool(name="brd", bufs=1))
    tmp_pool = ctx.enter_context(tc.tile_pool(name="tmp", bufs=1))
    vm_pool = ctx.enter_context(tc.tile_pool(name="vm", bufs=1))
    acc_pool = ctx.enter_context(tc.tile_pool(name="acc", bufs=1))
    fl_pool = ctx.enter_context(tc.tile_pool(name="fl", bufs=1))
    oacc_pool = ctx.enter_context(tc.tile_pool(name="oacc", bufs=1))
    c2_pool = ctx.enter_context(tc.tile_pool(name="c2", bufs=1))
    ps1 = ctx.enter_context(tc.tile_pool(name="ps1", bufs=3, space="PSUM"))
    psf = ctx.enter_context(tc.tile_pool(name="psf", bufs=1, space="PSUM"))
    ps2 = ctx.enter_context(tc.tile_pool(name="ps2", bufs=2, space="PSUM"))

    def mk(pool, shape, dtype, tag):
        return pool.tile(shape, dtype, tag=tag, name=tag)

    # channel-major border tiles, one per conv1 row-tile
    borderT = [mk(brd_pool, [C1_OUT[t], 16 * BW], f16, f"bT{t}")
               for t in range(3)]

    ones_c = mk(fl_pool, [128, 1], f32, "ones")
    nc.vector.memset(ones_c[:, :], 1.0)
    flp = [mk(psf, [1, 16], f32, f"flp{t}") for t in range(3)]
    flags_i = mk(fl_pool, [1, 16], i32, "flagsi")
    fall = mk(fl_pool, [1, 16], f32, "fall")
    fsb = mk(fl_pool, [1, 48], f32, "fsb")

    # ---- per conv1 tile: conv1 (fp8 DoubleRow), spikes, border logic ------
    for t in range(3 if STAGE >= 1 else 0):
        rows = C1_OUT[t]
        # orientation-batched spike tiles: slice o at cols [512o, 512o+512)
        SPE = mk(spk_pool, [rows, OW], bf16, "SPE")  # pol0 even ch (pe)
        SPO = mk(spk_pool, [rows, OW], bf16, "SPO")  # pol0 odd ch (po)
        SNE = mk(spk_pool, [rows, OW], bf16, "SNE")  # pol1 even ch (ne)
        SNO = mk(spk_pool, [rows, OW], bf16, "SNO")  # pol1 odd ch (no)
        CPO = mk(spk_pool, [rows, OW], bf16, "CPO")  # conv values via ACT
        CNO = mk(spk_pool, [rows, OW], bf16, "CNO")

        def spike_from(psum_ap, ch, pol, rows=None):
            o2, par = divmod(ch, 2)
            sl = slice(W * o2, W * o2 + W)
            if par == 0:
                dst = SPE if pol == 0 else SNE
                nc.vector.tensor_single_scalar(dst[:, sl], psum_ap, 1.0,
                                               AluOpType.is_ge)
            else:
                # odd channels: ACT copies the conv values, compare later
                cdst = CPO if pol == 0 else CNO
                nc.scalar.copy(cdst[:, sl], psum_ap)

        xt = []
        for pol in range(2):
            xx = mk(x_pool, [C1_IN[t], X8W], f8, f"x{pol}")
            nc.gpsimd.memset(xx[:, :], 0.0)
            nc.sync.dma_start(
                xx[:, 0:XW],
                x8_d[pol, C1_BASE[t]:C1_BASE[t] + C1_IN[t], 0:XW])
            nc.sync.dma_start(
                xx[:, DUP:DUP + XW],
                x8_d[pol, C1_BASE[t]:C1_BASE[t] + C1_IN[t], 0:XW])
            xt.append(xx)
        K = C1_IN[t]
        for ch in range(8):
            pp = [mk(ps1, [rows, W], f32, "c1") for _ in range(2)]
            for p in range(NPAIR):
                lhsT = _c1_lhsT(bandsB_t, ch, p, K, rows)
                for pol in range(2):
                    nc.tensor.matmul(
                        pp[pol][:, :], lhsT, _c1_rhs(xt[pol], p, K),
                        start=(p == 0), stop=(p == NPAIR - 1),
                        perf_mode=mybir.MatmulPerfMode.DoubleRow)
            for pol in range(2):
                spike_from(pp[pol][:, :], ch, pol)

        # batched compares for the odd channels
        nc.vector.tensor_single_scalar(SPO[:, :], CPO[:, :], 1.0,
                                       AluOpType.is_ge)
        nc.vector.tensor_single_scalar(SNO[:, :], CNO[:, :], 1.0,
                                       AluOpType.is_ge)

        if STAGE < 2:
            continue

        vm_t = mk(vm_pool, [rows, W], f32, "vm")
        nc.sync.dma_start(vm_t[:], vmap_d[C1_BASE[t]:C1_BASE[t] + rows, :])
        w1 = mk(vm_pool, [rows, W], bf16, "w1")
        nc.gpsimd.tensor_single_scalar(w1[:], vm_t[:], 1.0, AluOpType.is_ge)

        # zero the x-halo edges of all 16 border planes (strided memsets)
        bT = borderT[t]
        nc.gpsimd.memset(
            AP(bT.tensor, bT[0:rows, 0:1].offset,
               [[16 * BW, rows], [BW, 16], [1, PG]]), 0.0)
        nc.gpsimd.memset(
            AP(bT.tensor, bT[0:rows, 0:1].offset + PG + W,
               [[16 * BW, rows], [BW, 16], [1, PG]]), 0.0)

        def T(tag):
            return mk(tmp_pool, [rows, OW], bf16, tag)

        # exact algebra, orientation-batched:
        #   e13 = pe+ne - (pe*no + ne*po); e24 = po+no - (pe*no + ne*po)
        #   diff = e13 - e24 = (pe+ne) - (po+no)   (products cancel)
        A1 = T("A1"); nc.vector.tensor_mul(A1[:], SPE[:], SNO[:])
        B1 = T("B1"); nc.vector.tensor_mul(B1[:], SNE[:], SPO[:])
        C1 = T("C1"); nc.vector.tensor_add(C1[:], SPE[:], SNE[:])
        C2 = T("C2"); nc.vector.tensor_add(C2[:], SPO[:], SNO[:])
        DIFF = T("DIFF"); nc.vector.tensor_sub(DIFF[:], C1[:], C2[:])
        D1 = T("D1"); nc.vector.tensor_add(D1[:], A1[:], B1[:])
        E13 = T("E13"); nc.vector.tensor_sub(E13[:], C1[:], D1[:])
        E24 = T("E24"); nc.vector.tensor_sub(E24[:], C2[:], D1[:])
        TP = T("TP")
        nc.scalar.activation(TP[:], DIFF[:], mybir.ActivationFunctionType.Abs)
        # tmax over the 4 orientations
        m01 = mk(tmp_pool, [rows, W], bf16, "m01")
        nc.vector.tensor_max(m01[:], TP[:, 0:W], TP[:, W:2 * W])
        m23 = mk(tmp_pool, [rows, W], bf16, "m23")
        nc.vector.tensor_max(m23[:], TP[:, 2 * W:3 * W], TP[:, 3 * W:4 * W])
        TMAX = mk(tmp_pool, [rows, W], bf16, "TMAX")
        nc.vector.tensor_max(TMAX[:], m01[:], m23[:])
        # wta gate with w1 folded in: wd2 = (tp==tmax)*diff*w1
        tmaxb = AP(TMAX.tensor, TMAX[0:rows, 0:1].offset,
                   [[W, rows], [0, 4], [1, W]])
        w1b = AP(w1.tensor, w1[0:rows, 0:1].offset,
                 [[W, rows], [0, 4], [1, W]])
        WTA = T("C2")  # reuse: C2 dead after E24
        nc.vector.tensor_tensor(WTA[:], TP[:], tmaxb, AluOpType.is_equal)
        WD = T("D1")   # reuse: D1 dead after E24
        nc.vector.tensor_mul(WD[:], WTA[:], DIFF[:])
        # winner gates with the vmap mask folded in:
        # b1p = (wd>=1)*w1, b1n = (wd<=-1)*w1   (w1 in {0,1})
        B1P = T("C1")  # reuse: C1 dead after E13
        nc.vector.scalar_tensor_tensor(B1P[:], WD[:], 1.0, w1b,
                                       AluOpType.is_ge, AluOpType.mult)
        B1N = T("A1")  # reuse: A1 dead after D1
        nc.vector.scalar_tensor_tensor(B1N[:], WD[:], -1.0, w1b,
                                       AluOpType.is_le, AluOpType.mult)
        # border products: plane 4o+k at cols (4o+k)*534 + [11, 523)
        for k, (gate, src) in enumerate([
                (B1P, E13), (B1P, E24), (B1N, E24), (B1N, E13)]):
            bsl = AP(bT.tensor, bT[0:rows, 0:1].offset + k * BW + PG,
                     [[16 * BW, rows], [4 * BW, 4], [1, W]])
            nc.vector.tensor_mul(bsl, gate[:], src[:])

        if STAGE < 3:
            continue
        # EXACT flags: ch0 planes (4o+0 / 4o+2) nonzero iff b1p_o / b1n_o
        # fires anywhere (wd>=1 implies e13>=1); ch1 planes (4o+1 / 4o+3)
        # summed directly.  acc cols ordered by plane: [p0|p1|p2|p3] x o.
        acc_t = mk(acc_pool, [rows, 16], f32, "acc")
        nc.vector.tensor_reduce(acc_t[:, 0:16:4], _obatch(B1P, rows),
                                axis=AX.X, op=AluOpType.add)
        nc.vector.tensor_reduce(acc_t[:, 2:16:4], _obatch(B1N, rows),
                                axis=AX.X, op=AluOpType.add)
        for ci, k in ((1, 1), (3, 3)):
            pl = AP(bT.tensor, bT[0:rows, 0:1].offset + k * BW + PG,
                    [[16 * BW, rows], [4 * BW, 4], [1, W]])
            nc.vector.tensor_reduce(acc_t[:, ci:16:4], pl,
                                    axis=AX.X, op=AluOpType.add)
        nc.tensor.matmul(flp[t][0:1, 0:16], ones_c[0:rows, :],
                         acc_t[:, :], start=True, stop=True)

    # ---- flags: per-plane nonzero counts; plane index 2q+j is already
    # (pair q, channel j), so the pair layout is the identity.
    if STAGE >= 3:
        for t in range(3):
            nc.scalar.copy(fsb[0:1, 16 * t:16 * t + 16], flp[t][0:1, :])
        nc.vector.tensor_add(fall[0:1, :], fsb[0:1, 0:16], fsb[0:1, 16:32])
        nc.vector.tensor_add(fall[0:1, :], fall[0:1, :], fsb[0:1, 32:48])
        nc.vector.tensor_copy(flags_i[0:1, :], fall[0:1, :])

    # ---- conv2, skipped per (pair, span) when the a-plane is all zero -----
    oacc = [mk(oacc_pool, [E_OUT[e], W], f32, f"oacc{e}") for e in range(3)]
    for e in range(3):
        nc.gpsimd.memset(oacc[e][:, :], 0.0)

    def _c2conv(ch, e, u, tag):
        """spike(conv2) of border plane ch on out-tile e with band u."""
        orows, krows = E_OUT[e], E_IN[e]
        pg = mk(ps2, [orows, W], f32, "c2")
        gb = bandsG_t[u]
        for dx in range(GK):
            nc.tensor.matmul(
                pg[:, :],
                gb[0:krows, dx * 106:dx * 106 + orows],
                borderT[e][0:krows, ch * BW + dx:ch * BW + dx + W],
                start=(dx == 0), stop=(dx == GK - 1))
        s = mk(c2_pool, [orows, W], bf16, tag)
        nc.vector.tensor_single_scalar(s[:], pg[:, :], 1.0, AluOpType.is_ge)
        return s

    if STAGE >= 4 and USE_SKIP:
        _, pair_flags = nc.values_load_multi_w_load_instructions(
            flags_i[0:1, 0:16], engines=[ET.PE, ET.DVE],
            skip_runtime_bounds_check=True)
    for q in range(8 if STAGE >= 4 else 0):
        o, pk = divmod(q, 2)
        ch0 = 4 * o + 2 * pk
        u = 2 * o + pk

        def IF(c, nm):
            return tc.If(c, name=nm) if USE_SKIP else nullcontext()
        with IF(pair_flags[2 * q] != 0 if USE_SKIP else 1, f"q{q}"):
            aa = []
            for e in range(3):
                a = _c2conv(ch0, e, u, f"a{e}")
                nc.vector.tensor_add(oacc[e][:, :], oacc[e][:, :], a[:])
                aa.append(a)
            with IF(pair_flags[2 * q + 1] != 0 if USE_SKIP else 1, f"qn{q}"):
                for e in range(3):
                    g = _c2conv(ch0 + 1, e, u, "g")
                    ag = mk(c2_pool, [E_OUT[0], W], bf16, "ag")[0:E_OUT[e], :]
                    nc.vector.tensor_mul(ag[:], aa[e][:], g[:])
                    nc.vector.tensor_sub(oacc[e][:, :], oacc[e][:, :], ag[:])

    for e in range(3):
        nc.sync.dma_start(out_d[E_BASE[e]:E_BASE[e] + E_OUT[e], :], oacc[e][:])


def _build_program(bandsB_np, bandsG_np, reps=1):
    from contextlib import ExitStack
    nc = bacc.Bacc("TRN2", target_bir_lowering=False, debug=False,
                   num_devices=N_CORES)
    x8_d = nc.dram_tensor("x8", [2, IN_ROWS, X8DW], f8, kind="ExternalInput").ap()
    vmap_d = nc.dram_tensor("vmap", [C1_ROWS, W], f32, kind="ExternalInput").ap()
    bandsB_d = nc.inline_tensor(bandsB_np, name="bandsB").ap()
    bandsG_d = nc.inline_tensor(bandsG_np, name="bandsG").ap()
    out_d = nc.dram_tensor("out", [HALF, W], f32, kind="ExternalOutput").ap()

    with tile.TileContext(nc) as tc:
        with ExitStack() as octx:
            band_pool = octx.enter_context(tc.tile_pool(name="bands", bufs=1))
            bandsB_t = band_pool.tile([128, 8 * NPAIR * 256], f8,
                                      tag="bB", name="bB")
            nc.sync.dma_start(bandsB_t[:], bandsB_d)
            bandsG_t = []
            for u in range(8):
                g = band_pool.tile([128, GK * 106], f16, tag=f"bG{u}",
                                   name=f"bG{u}")
                nc.sync.dma_start(g[:], bandsG_d[u])
                bandsG_t.append(g)
            if reps == 1:
                with ExitStack() as ctx:
                    _emit(nc, tc, ctx, x8_d, vmap_d, bandsB_t,
                          bandsG_t, out_d)
            else:
                with tc.For_i(0, reps, 1,
                              staggered_reset=os.environ.get("K_SR", "1") == "1"):
                    with ExitStack() as ctx:
                        _emit(nc, tc, ctx, x8_d, vmap_d, bandsB_t,
                              bandsG_t, out_d)
    nc.compile()
    return nc


_PROGRAM_CACHE = {}


def kernel(inp, W_border, W_group):
    in_maps = _prep_inputs(inp)
    bandsB_np, bandsG_np = _make_bands(W_border, W_group)
    key = (bandsB_np.tobytes(), bandsG_np.tobytes())
    if _PROGRAM_CACHE.get("key") != key:
        _PROGRAM_CACHE["nc"] = _build_program(bandsB_np, bandsG_np)
        _PROGRAM_CACHE["key"] = key
    res = run_bass_kernel_spmd(_PROGRAM_CACHE["nc"], in_maps, list(range(N_CORES)))
    out = np.empty((4, H, W), dtype=np.float32)
    for r in range(N_CORES):
        b, half = divmod(r, 2)
        out[b, HALF * half:HALF * (half + 1), :] = res.results[r]["out"]
    return out
